# revision 19
# baseline (speedup 1.0000x reference)
"""Trainium2 Bass kernel for nn_Net_2491081031714.

Math per row x (784 f32):
  s_k = sum_{j>=k} x_j^2 (k=0..9), r = sqrt(s_0)
  theta_k = arccos(x_k / sqrt(s_k)) = pi/2 - arctan(x_k / sqrt(s_{k+1}))
  th_k = relu(relu(theta_k + rot1_k) + rot2_k) + rot3_k
       = max(theta_k + A_k, B_k),  A = rot1+rot2+rot3, B = max(rot2,0)+rot3
  r3 = r * relu(relu(scale1)*scale2)*scale3
  cart = polar_to_cartesian(r3, th)  (10 values); out = softmax(cart)

Only th_0..th_8 survive (polar_linear out_dim=10 truncates), so the heavy
work is the per-row sum of squares of cols 9..783, streamed in fp16 over
two DMA rings (Sync HWDGE + GpSimd SWDGE) and consumed tile-at-a-time by
the Scalar (ACT Square+accum) and Vector (STT+accum) engines, whose
accum-reads drop s9 straight into the suffix-scan's data1 slot 0.
rsqrt is a Quake seed + one Halley step; theta uses the exact arctan
identity; the relu chain folds to max(A'-arctan, B) with A'/B pre-wrapped
by a common 2pi multiple on the host so one add_range_wrap replaces the
3-op magic-rounding wrap; exp's table set is prefetched by a pinned dummy
ACT Exp so the switch overlaps the DVE tail.

Sharding: pure batch data-parallel over 8 cores (2048 rows each).
"""

import numpy as np

import concourse.bacc as bacc
import concourse.tile as tile
from concourse import mybir
from concourse.bass_utils import run_bass_kernel_spmd

AF = mybir.ActivationFunctionType
OP = mybir.AluOpType
F32 = mybir.dt.float32
I32 = mybir.dt.int32
F16 = mybir.dt.float16

B, N = 16384, 784
NCORES = 8
ROWS = B // NCORES          # 2048
P = 128
NT = ROWS // P              # 16 row-tiles per core
K = 9                       # thetas that matter
NO = 10                     # output classes

TWO_PI = 6.283185307179586
HALF_PI = 1.5707963267948966
RSQRT_MAGIC = 0x5F3759DF    # Quake rsqrt seed constant

# pc (host-prepared params) column layout
PC_C = 0                    # scale product
PC_A = 1                    # wrapped pi/2 + rot1+rot2+rot3, reversed k [9]
PC_B = PC_A + K             # wrapped max(rot2,0)+rot3, reversed k     [9]
PC_W = PC_B + K

# DMA groups (start_tile, n_tiles, ring): ring 0 = Sync HWDGE, ring 1 =
# GpSimd SWDGE, ring 2 = Scalar HWDGE.  Aggregate DMA tops out at ~300 GB/s
# regardless of ring count, so the goal is supply ORDER: single-tile groups
# round-robin across the three rings land close to tile order, keeping both
# compute engines fed even when one ring runs slow.
GROUPS = [(t, 1, (0, 2, 1)[t % 3]) for t in range(NT)]
# whole-tile engine assignment: measured DVE 1051ns/tile vs ACT 1222ns/tile
# (incl. accum-read) -> 9/7, queues in plain tile order.
DVE_ORDER = (0, 2, 4, 6, 8, 10, 12, 14, 15)
ACT_ORDER = (1, 3, 5, 7, 9, 11, 13)


def _build():
    nc = bacc.Bacc("TRN2", target_bir_lowering=False, debug=False)
    x = nc.dram_tensor("x", [ROWS, N], F16, kind="ExternalInput")
    x9 = nc.dram_tensor("x9", [ROWS, K], F32, kind="ExternalInput")
    pc = nc.dram_tensor("pc", [P, PC_W], F32, kind="ExternalInput")
    y = nc.dram_tensor("y", [ROWS, NO], F32, kind="ExternalOutput")

    # row <-> (partition, slot) mapping: row = 16*p + t, so a group's rows
    # are contiguous per partition (up to 6272B descriptors)
    xg_view = x.rearrange("(p t) n -> p t n", p=P)              # [P, NT, N]
    x9_view = x9.rearrange("(p t) k -> p t k", p=P)             # [P, NT, K]
    y_view = y.rearrange("(p t) k -> p t k", p=P)               # [P, NT, NO]

    with tile.TileContext(nc) as tc:
        with (
            tc.tile_pool(name="xpool", bufs=1) as xpool,
            tc.tile_pool(name="sing", bufs=1) as sing,
        ):
            # ACT table preload: first ACTIVATE is a Sin so trig_and_small
            # (sin+arctan+square+relu) loads under the DMA ramp.
            warm = sing.tile([P, 1], F32)
            nc.vector.memset(warm[:], 0.0)
            nc.scalar.activation(warm[:], warm[:], AF.Sin)

            xg = [xpool.tile([P, nt, N], F16, name=f"xg{g}", tag=f"xg{g}")
                  for g, (t0, nt, ring) in enumerate(GROUPS)]
            pct = sing.tile([P, PC_W], F32)
            x9n = sing.tile([P, NT, K], F32)      # x[:, 0:9] natural order
            engs = {0: nc.sync, 1: nc.gpsimd, 2: nc.scalar}
            for g, (t0, nt, ring) in enumerate(GROUPS):
                engs[ring].dma_start(xg[g][:], xg_view[:, t0:t0 + nt, :])
            # pc/x9 are tiny and first needed at ~20us (epilogue), so they go
            # behind all x groups on the slow SWDGE ring
            nc.gpsimd.dma_start(pct[:], pc[:])
            nc.gpsimd.dma_start(x9n[:], x9_view)

            # persistent small tiles
            d0s = sing.tile([P, NT, NO], F32)     # scan data0 for suffix sums
            d1s = sing.tile([P, NT, NO], F32)     # scan data1 for suffix sums
            scnb = sing.tile([P, NT, NO], F32)    # [0, sin_0..sin_8] per block
            d1p = sing.tile([P, NT, NO], F32)     # scan data1 for cumprod
            sqa = sing.tile([P, N - K], F16)      # ACT squares scratch (dead)
            sqd = sing.tile([P, N - K], F16)      # DVE squares scratch (dead)

            nc.gpsimd.memset(d0s[:], 1.0)
            nc.gpsimd.memset(d0s[:, :, 0:1], 0.0)
            nc.gpsimd.memset(scnb[:, :, 0:1], 0.0)
            nc.gpsimd.memset(d1p[:, :, 1:], 0.0)
            # squares of the first 9 cols (reversed order) on the idle gpsimd
            nc.gpsimd.tensor_mul(d1s[:, :, 1:NO], x9n[:, :, ::-1],
                                 x9n[:, :, ::-1])

            # ---- main streaming square+reduce: each tile goes whole to one
            #      engine; the accum-read drops s9 straight into the scan's
            #      data1 slot 0 (no separate seed add) ----
            tile_group = {}
            for g, (t0, nt, ring) in enumerate(GROUPS):
                for j in range(nt):
                    tile_group[t0 + j] = (g, j)
            for t in DVE_ORDER:
                g, j = tile_group[t]
                nc.vector.scalar_tensor_tensor(
                    out=sqd[:], in0=xg[g][:, j, K:N], scalar=1.0,
                    in1=xg[g][:, j, K:N], op0=OP.mult, op1=OP.mult,
                    accum_out=d1s[:, t, 0:1],
                )
            for t in ACT_ORDER:
                g, j = tile_group[t]
                nc.scalar.activation(
                    out=sqa[:], in_=xg[g][:, j, K:N], func=AF.Square,
                    accum_out=d1s[:, t, 0:1],
                )

            # ---- epilogue (batched over all 16 row-tiles) ----
            ep = sing

            # suffix-sum scan: S[:, :, m] = s_{9-m} for m=0..9 (m=9 -> s_0)
            S = ep.tile([P, NT, NO], F32)
            nc.vector.tensor_tensor_scan(
                out=S[:].rearrange("p b k -> p (b k)"),
                data0=d0s[:].rearrange("p b k -> p (b k)"),
                data1=d1s[:].rearrange("p b k -> p (b k)"),
                initial=0.0, op0=OP.mult, op1=OP.add,
            )

            # rsqrt of all 10 suffix sums: Quake seed + one Halley step
            # y1 = y0*(1.875 - 1.25*w + 0.375*w^2), w = S*y0^2  (~1e-4 rel)
            sbits = S[:].bitcast(I32)
            y0i = ep.tile([P, NT, NO], I32)
            nc.vector.tensor_scalar(out=y0i[:], in0=sbits, scalar1=1, scalar2=-1,
                                    op0=OP.arith_shift_right, op1=OP.bitwise_xor)
            nc.vector.tensor_scalar(out=y0i[:], in0=y0i[:],
                                    scalar1=RSQRT_MAGIC + 1, scalar2=None,
                                    op0=OP.add)
            yv = y0i[:].bitcast(F32)
            aa = ep.tile([P, NT, NO], F32)
            ww = ep.tile([P, NT, NO], F32)
            inv = ep.tile([P, NT, NO], F32)
            dacc = ep.tile([P, 1], F32)   # dummy accum for affine_mul_reduce
            nc.vector.tensor_mul(aa[:], yv, yv)
            nc.vector.tensor_mul(ww[:], aa[:], S[:])
            nc.vector.affine_mul_reduce(out=aa[:], accum_out=dacc[:],
                                        in0=ww[:], in1=ww[:], scale=0.375,
                                        bias=-1.25)
            nc.vector.affine_mul_reduce(out=inv[:], accum_out=dacc[:],
                                        in0=aa[:], in1=yv, scale=1.0,
                                        bias=1.875)
            # inv[:, :, m] = rsqrt(s_{9-m})

            # r3 = c * s_0 * rsqrt(s_0) -> cumprod scan seed, on gpsimd so it
            # overlaps the DVE arctan path
            nc.gpsimd.tensor_mul(d1p[:, :, 0:1], S[:, :, NO - 1:NO],
                                 inv[:, :, NO - 1:NO])
            nc.gpsimd.tensor_scalar(out=d1p[:, :, 0:1], in0=d1p[:, :, 0:1],
                                    scalar1=pct[:, PC_C:PC_C + 1],
                                    scalar2=None, op0=OP.mult)

            # theta_k = pi/2 - arctan(x_k * rsqrt(s_{k+1})); reversed order j
            # uses w_j = x_{8-j} * rsqrt(s_{9-j}) = x9n_rev * inv[:, :, 0:9]
            w9 = ep.tile([P, NT, K], F32)
            nc.vector.tensor_mul(w9[:], x9n[:, :, ::-1], inv[:, :, 0:K])
            at = ep.tile([P, NT, K], F32)
            nc.scalar.activation(at[:], w9[:], AF.Arctan)

            av = pct[:, PC_A:PC_A + K].unsqueeze(1).broadcast_to([P, NT, K])
            bv = pct[:, PC_B:PC_B + K].unsqueeze(1).broadcast_to([P, NT, K])

            th = ep.tile([P, NT, K], F32)
            # th3 = max(A' - arctan, B); A'/B are pre-wrapped by a common 2pi
            # multiple per k, so |th3| < 3pi and one range-wrap suffices
            nc.vector.scalar_tensor_tensor(out=th[:], in0=at[:], scalar=-1.0,
                                           in1=av, op0=OP.mult, op1=OP.add)
            nc.vector.tensor_tensor(out=th[:], in0=th[:], in1=bv, op=OP.max)
            thp = ep.tile([P, NT, K], F32)
            nc.vector.add_range_wrap(out=thp[:], in_=th[:], shift=0.0,
                                     bound=np.pi, period=TWO_PI)

            # sins in natural order into scnb slots 1..9 (thp is rev order)
            nc.scalar.activation(scnb[:, :, 1:NO], thp[:, :, ::-1], AF.Sin)
            # cos (rev order) = sin(wrap(thp + pi/2)) via add_range_wrap
            y2 = ep.tile([P, NT, K], F32)
            nc.vector.add_range_wrap(out=y2[:], in_=thp[:], shift=HALF_PI,
                                     bound=np.pi, period=TWO_PI)
            ccr = ep.tile([P, NT, K], F32)
            nc.scalar.activation(ccr[:], y2[:], AF.Sin)

            # prefetch the exp table set while the DVE runs the cumprod tail.
            # The input view of ccr pins this AFTER the sins in the schedule
            # (a free-floating dummy would be hoisted early and thrash tables).
            nc.scalar.activation(warm[:], ccr[:, 0, 0:1], AF.Exp)

            # cumprod scan: PP[:, :, m] = r3 * prod_{i<m} sin_i
            PP = ep.tile([P, NT, NO], F32)
            nc.vector.tensor_tensor_scan(
                out=PP[:].rearrange("p b k -> p (b k)"),
                data0=scnb[:].rearrange("p b k -> p (b k)"),
                data1=d1p[:].rearrange("p b k -> p (b k)"),
                initial=0.0, op0=OP.mult, op1=OP.add,
            )

            lg = ep.tile([P, NT, NO], F32)
            # the two small slot products go to gpsimd, in parallel with the
            # DVE's 8-slot product
            nc.gpsimd.tensor_mul(lg[:, :, 0:1], PP[:, :, K - 1:K],
                                 ccr[:, :, 0:1])
            nc.gpsimd.tensor_mul(lg[:, :, 1:2], PP[:, :, K - 1:K],
                                 scnb[:, :, NO - 1:NO])
            nc.vector.tensor_mul(lg[:, :, 2:NO], PP[:, :, 7::-1], ccr[:, :, 1:K])

            # softmax without max-sub (|logits| <= ~45, f32-safe)
            E = ep.tile([P, NT, NO], F32)
            nc.scalar.activation(E[:], lg[:], AF.Exp)
            ds = ep.tile([P, NT], F32)
            nc.vector.tensor_reduce(out=ds[:], in_=E[:], axis=mybir.AxisListType.X,
                                    op=OP.add)
            dinv = ep.tile([P, NT], F32)
            nc.vector.reciprocal_approx_fast(dinv[:], ds[:])
            out = ep.tile([P, NT, NO], F32)
            H = NT // 2
            nc.vector.tensor_mul(
                out[:, 0:H, :], E[:, 0:H, :],
                dinv[:, 0:H].unsqueeze(2).broadcast_to([P, H, NO]))
            nc.sync.dma_start(y_view[:, 0:H, :], out[:, 0:H, :])
            nc.vector.tensor_mul(
                out[:, H:, :], E[:, H:, :],
                dinv[:, H:].unsqueeze(2).broadcast_to([P, NT - H, NO]))
            nc.sync.dma_start(y_view[:, H:, :], out[:, H:, :])

    nc.compile()
    return nc


_NC = None


def _get_nc():
    global _NC
    if _NC is None:
        _NC = _build()
    return _NC


def _host_params(scale1, rot1, scale2, rot2, scale3, rot3):
    c = max(max(float(scale1[0]), 0.0) * float(scale2[0]), 0.0) * float(scale3[0])
    rev = np.arange(K - 1, -1, -1)
    r1 = rot1[:K].astype(np.float64)
    r2 = rot2[:K].astype(np.float64)
    r3 = rot3[:K].astype(np.float64)
    a = np.pi / 2 + r1 + r2 + r3
    b = np.maximum(r2, 0.0) + r3
    # wrap A'/B by a common per-k multiple of 2pi (preserves the max branch
    # and sin values) so |max(A'-at, B)| < 3pi and one add_range_wrap wraps it
    s = TWO_PI * np.round((a + b) / 2.0 / TWO_PI)
    a -= s
    b -= s
    assert np.abs(a).max() + np.pi / 2 < 3 * np.pi - 0.2, "ARW bound violated"
    assert np.abs(b).max() < 3 * np.pi - 0.2, "ARW bound violated"
    row = np.zeros((PC_W,), np.float64)
    row[PC_C] = c
    row[PC_A:PC_A + K] = a[rev]
    row[PC_B:PC_B + K] = b[rev]
    return np.tile(row.astype(np.float32)[None, :], (P, 1))


def kernel(x, scale1, rot1, scale2, rot2, scale3, rot3, _trace=False):
    nc = _get_nc()
    pc = _host_params(scale1, rot1, scale2, rot2, scale3, rot3)
    x = np.ascontiguousarray(x, dtype=np.float32)
    xh = x.astype(np.float16)
    x9h = np.ascontiguousarray(x[:, 0:K])
    in_maps = [
        {"x": xh[c * ROWS:(c + 1) * ROWS], "pc": pc,
         "x9": x9h[c * ROWS:(c + 1) * ROWS]} for c in range(NCORES)
    ]
    res = run_bass_kernel_spmd(nc, in_maps, core_ids=list(range(NCORES)),
                               trace=_trace)
    out = np.concatenate([res.results[c]["y"] for c in range(NCORES)], axis=0)
    if _trace:
        return out, res
    return out


# revision 21
# speedup vs baseline: 1.0916x; 1.0916x over previous
"""Trainium2 Bass kernel for nn_Net_2491081031714.

Math per row x (784 f32):
  s_k = sum_{j>=k} x_j^2 (k=0..9), r = sqrt(s_0)
  theta_k = arccos(x_k / sqrt(s_k)) = pi/2 - arctan(x_k / sqrt(s_{k+1}))
  th_k = relu(relu(theta_k + rot1_k) + rot2_k) + rot3_k
       = max(theta_k + A_k, B_k),  A = rot1+rot2+rot3, B = max(rot2,0)+rot3
  r3 = r * relu(relu(scale1)*scale2)*scale3
  cart = polar_to_cartesian(r3, th)  (10 values); out = softmax(cart)

Only th_0..th_8 survive (polar_linear out_dim=10 truncates), so the heavy
work is the per-row sum of squares of cols 9..783, streamed in fp16 over
two DMA rings (Sync HWDGE + GpSimd SWDGE) and consumed tile-at-a-time by
the Scalar (ACT Square+accum) and Vector (STT+accum) engines, whose
accum-reads drop s9 straight into the suffix-scan's data1 slot 0.
rsqrt is a Quake seed + one Halley step; theta uses the exact arctan
identity; the relu chain folds to max(A'-arctan, B) with A'/B pre-wrapped
by a common 2pi multiple on the host so one add_range_wrap replaces the
3-op magic-rounding wrap; exp's table set is prefetched by a pinned dummy
ACT Exp so the switch overlaps the DVE tail.

Sharding: pure batch data-parallel over 8 cores (2048 rows each).
"""

import numpy as np

import concourse.bacc as bacc
import concourse.tile as tile
from concourse import mybir
from concourse.bass_utils import run_bass_kernel_spmd

AF = mybir.ActivationFunctionType
OP = mybir.AluOpType
F32 = mybir.dt.float32
I32 = mybir.dt.int32
F16 = mybir.dt.float16

B, N = 16384, 784
NCORES = 8
ROWS = B // NCORES          # 2048
P = 128
NT = ROWS // P              # 16 row-tiles per core
K = 9                       # thetas that matter
NO = 10                     # output classes

TWO_PI = 6.283185307179586
HALF_PI = 1.5707963267948966
RSQRT_MAGIC = 0x5F3759DF    # Quake rsqrt seed constant

# pc (host-prepared params) column layout
PC_C = 0                    # scale product
PC_A = 1                    # wrapped pi/2 + rot1+rot2+rot3, reversed k [9]
PC_B = PC_A + K             # wrapped max(rot2,0)+rot3, reversed k     [9]
PC_W = PC_B + K

# DMA groups (start_tile, n_tiles, ring): ring 0 = Sync HWDGE, ring 1 =
# GpSimd SWDGE.  The rings run at similar pace when concurrent (~150 GB/s
# each), so strict alternation lands the 2-tile groups in tile order and
# splits the stream 8/8 so neither ring becomes an 11us serial chain.
GROUPS = [(0, 2, 0), (2, 2, 1), (4, 2, 0), (6, 2, 1), (8, 2, 0), (10, 2, 1),
          (12, 2, 0), (14, 2, 1)]
# whole-tile engine assignment: measured DVE 1051ns/tile vs ACT 1222ns/tile
# (incl. accum-read) -> 9/7, queues in plain tile order.
DVE_ORDER = (0, 1, 2, 4, 6, 8, 10, 12, 14)
ACT_ORDER = (3, 5, 7, 9, 11, 13, 15)


def _build():
    nc = bacc.Bacc("TRN2", target_bir_lowering=False, debug=False)
    x = nc.dram_tensor("x", [ROWS, N], F16, kind="ExternalInput")
    x9 = nc.dram_tensor("x9", [ROWS, K], F32, kind="ExternalInput")
    pc = nc.dram_tensor("pc", [P, PC_W], F32, kind="ExternalInput")
    y = nc.dram_tensor("y", [ROWS, NO], F32, kind="ExternalOutput")

    # row <-> (partition, slot) mapping: row = 16*p + t, so a group's rows
    # are contiguous per partition (up to 6272B descriptors)
    xg_view = x.rearrange("(p t) n -> p t n", p=P)              # [P, NT, N]
    x9_view = x9.rearrange("(p t) k -> p t k", p=P)             # [P, NT, K]
    y_view = y.rearrange("(p t) k -> p t k", p=P)               # [P, NT, NO]

    with tile.TileContext(nc) as tc:
        with (
            tc.tile_pool(name="xpool", bufs=1) as xpool,
            tc.tile_pool(name="sing", bufs=1) as sing,
        ):
            # ACT table preload: first ACTIVATE is a Sin so trig_and_small
            # (sin+arctan+square+relu) loads under the DMA ramp.
            warm = sing.tile([P, 1], F32)
            nc.vector.memset(warm[:], 0.0)
            nc.scalar.activation(warm[:], warm[:], AF.Sin)

            xg = [xpool.tile([P, nt, N], F16, name=f"xg{g}", tag=f"xg{g}")
                  for g, (t0, nt, ring) in enumerate(GROUPS)]
            pct = sing.tile([P, PC_W], F32)
            x9n = sing.tile([P, NT, K], F32)      # x[:, 0:9] natural order
            for g, (t0, nt, ring) in enumerate(GROUPS):
                eng = nc.sync if ring == 0 else nc.gpsimd
                eng.dma_start(xg[g][:], xg_view[:, t0:t0 + nt, :])
            # pc/x9 are tiny and first needed at ~20us (epilogue), so they go
            # behind all x groups on the slow SWDGE ring
            nc.gpsimd.dma_start(pct[:], pc[:])
            nc.gpsimd.dma_start(x9n[:], x9_view)

            # persistent small tiles
            d0s = sing.tile([P, NT, NO], F32)     # scan data0 for suffix sums
            d1s = sing.tile([P, NT, NO], F32)     # scan data1 for suffix sums
            scnb = sing.tile([P, NT, NO], F32)    # [0, sin_0..sin_8] per block
            d1p = sing.tile([P, NT, NO], F32)     # scan data1 for cumprod
            sqa = sing.tile([P, N - K], F16)      # ACT squares scratch (dead)
            sqd = sing.tile([P, N - K], F16)      # DVE squares scratch (dead)

            nc.gpsimd.memset(d0s[:], 1.0)
            nc.gpsimd.memset(d0s[:, :, 0:1], 0.0)
            nc.gpsimd.memset(scnb[:, :, 0:1], 0.0)
            nc.gpsimd.memset(d1p[:, :, 1:], 0.0)
            # squares of the first 9 cols (reversed order) on the idle gpsimd
            nc.gpsimd.tensor_mul(d1s[:, :, 1:NO], x9n[:, :, ::-1],
                                 x9n[:, :, ::-1])

            # ---- main streaming square+reduce: each tile goes whole to one
            #      engine; the accum-read drops s9 straight into the scan's
            #      data1 slot 0 (no separate seed add) ----
            tile_group = {}
            for g, (t0, nt, ring) in enumerate(GROUPS):
                for j in range(nt):
                    tile_group[t0 + j] = (g, j)
            for t in DVE_ORDER:
                g, j = tile_group[t]
                nc.vector.scalar_tensor_tensor(
                    out=sqd[:], in0=xg[g][:, j, K:N], scalar=1.0,
                    in1=xg[g][:, j, K:N], op0=OP.mult, op1=OP.mult,
                    accum_out=d1s[:, t, 0:1],
                )
            for t in ACT_ORDER:
                g, j = tile_group[t]
                nc.scalar.activation(
                    out=sqa[:], in_=xg[g][:, j, K:N], func=AF.Square,
                    accum_out=d1s[:, t, 0:1],
                )

            # ---- epilogue (batched over all 16 row-tiles) ----
            ep = sing

            # suffix-sum scan: S[:, :, m] = s_{9-m} for m=0..9 (m=9 -> s_0)
            S = ep.tile([P, NT, NO], F32)
            nc.vector.tensor_tensor_scan(
                out=S[:].rearrange("p b k -> p (b k)"),
                data0=d0s[:].rearrange("p b k -> p (b k)"),
                data1=d1s[:].rearrange("p b k -> p (b k)"),
                initial=0.0, op0=OP.mult, op1=OP.add,
            )

            # rsqrt of all 10 suffix sums: Quake seed + one Halley step
            # y1 = y0*(1.875 - 1.25*w + 0.375*w^2), w = S*y0^2  (~1e-4 rel)
            sbits = S[:].bitcast(I32)
            y0i = ep.tile([P, NT, NO], I32)
            nc.vector.tensor_scalar(out=y0i[:], in0=sbits, scalar1=1, scalar2=-1,
                                    op0=OP.arith_shift_right, op1=OP.bitwise_xor)
            nc.vector.tensor_scalar(out=y0i[:], in0=y0i[:],
                                    scalar1=RSQRT_MAGIC + 1, scalar2=None,
                                    op0=OP.add)
            yv = y0i[:].bitcast(F32)
            aa = ep.tile([P, NT, NO], F32)
            ww = ep.tile([P, NT, NO], F32)
            inv = ep.tile([P, NT, NO], F32)
            dacc = ep.tile([P, 1], F32)   # dummy accum for affine_mul_reduce
            nc.vector.tensor_mul(aa[:], yv, yv)
            nc.vector.tensor_mul(ww[:], aa[:], S[:])
            nc.vector.affine_mul_reduce(out=aa[:], accum_out=dacc[:],
                                        in0=ww[:], in1=ww[:], scale=0.375,
                                        bias=-1.25)
            nc.vector.affine_mul_reduce(out=inv[:], accum_out=dacc[:],
                                        in0=aa[:], in1=yv, scale=1.0,
                                        bias=1.875)
            # inv[:, :, m] = rsqrt(s_{9-m})

            # r3 = c * s_0 * rsqrt(s_0) -> cumprod scan seed, on gpsimd so it
            # overlaps the DVE arctan path
            nc.gpsimd.tensor_mul(d1p[:, :, 0:1], S[:, :, NO - 1:NO],
                                 inv[:, :, NO - 1:NO])
            nc.gpsimd.tensor_scalar(out=d1p[:, :, 0:1], in0=d1p[:, :, 0:1],
                                    scalar1=pct[:, PC_C:PC_C + 1],
                                    scalar2=None, op0=OP.mult)

            # theta_k = pi/2 - arctan(x_k * rsqrt(s_{k+1})); reversed order j
            # uses w_j = x_{8-j} * rsqrt(s_{9-j}) = x9n_rev * inv[:, :, 0:9]
            w9 = ep.tile([P, NT, K], F32)
            nc.vector.tensor_mul(w9[:], x9n[:, :, ::-1], inv[:, :, 0:K])
            at = ep.tile([P, NT, K], F32)
            nc.scalar.activation(at[:], w9[:], AF.Arctan)

            av = pct[:, PC_A:PC_A + K].unsqueeze(1).broadcast_to([P, NT, K])
            bv = pct[:, PC_B:PC_B + K].unsqueeze(1).broadcast_to([P, NT, K])

            th = ep.tile([P, NT, K], F32)
            # th3 = max(A' - arctan, B); A'/B are pre-wrapped by a common 2pi
            # multiple per k, so |th3| < 3pi and one range-wrap suffices
            nc.vector.scalar_tensor_tensor(out=th[:], in0=at[:], scalar=-1.0,
                                           in1=av, op0=OP.mult, op1=OP.add)
            nc.vector.tensor_tensor(out=th[:], in0=th[:], in1=bv, op=OP.max)
            thp = ep.tile([P, NT, K], F32)
            nc.vector.add_range_wrap(out=thp[:], in_=th[:], shift=0.0,
                                     bound=np.pi, period=TWO_PI)

            # sins in natural order into scnb slots 1..9 (thp is rev order)
            nc.scalar.activation(scnb[:, :, 1:NO], thp[:, :, ::-1], AF.Sin)
            # cos (rev order) = sin(wrap(thp + pi/2)) via add_range_wrap
            y2 = ep.tile([P, NT, K], F32)
            nc.vector.add_range_wrap(out=y2[:], in_=thp[:], shift=HALF_PI,
                                     bound=np.pi, period=TWO_PI)
            ccr = ep.tile([P, NT, K], F32)
            nc.scalar.activation(ccr[:], y2[:], AF.Sin)

            # prefetch the exp table set while the DVE runs the cumprod tail.
            # The input view of ccr pins this AFTER the sins in the schedule
            # (a free-floating dummy would be hoisted early and thrash tables).
            nc.scalar.activation(warm[:], ccr[:, 0, 0:1], AF.Exp)

            # cumprod scan: PP[:, :, m] = r3 * prod_{i<m} sin_i
            PP = ep.tile([P, NT, NO], F32)
            nc.vector.tensor_tensor_scan(
                out=PP[:].rearrange("p b k -> p (b k)"),
                data0=scnb[:].rearrange("p b k -> p (b k)"),
                data1=d1p[:].rearrange("p b k -> p (b k)"),
                initial=0.0, op0=OP.mult, op1=OP.add,
            )

            lg = ep.tile([P, NT, NO], F32)
            # the two small slot products go to gpsimd, in parallel with the
            # DVE's 8-slot product
            nc.gpsimd.tensor_mul(lg[:, :, 0:1], PP[:, :, K - 1:K],
                                 ccr[:, :, 0:1])
            nc.gpsimd.tensor_mul(lg[:, :, 1:2], PP[:, :, K - 1:K],
                                 scnb[:, :, NO - 1:NO])
            nc.vector.tensor_mul(lg[:, :, 2:NO], PP[:, :, 7::-1], ccr[:, :, 1:K])

            # softmax without max-sub (|logits| <= ~45, f32-safe)
            E = ep.tile([P, NT, NO], F32)
            nc.scalar.activation(E[:], lg[:], AF.Exp)
            ds = ep.tile([P, NT], F32)
            nc.vector.tensor_reduce(out=ds[:], in_=E[:], axis=mybir.AxisListType.X,
                                    op=OP.add)
            dinv = ep.tile([P, NT], F32)
            nc.vector.reciprocal_approx_fast(dinv[:], ds[:])
            out = ep.tile([P, NT, NO], F32)
            H = NT // 2
            nc.vector.tensor_mul(
                out[:, 0:H, :], E[:, 0:H, :],
                dinv[:, 0:H].unsqueeze(2).broadcast_to([P, H, NO]))
            nc.sync.dma_start(y_view[:, 0:H, :], out[:, 0:H, :])
            nc.vector.tensor_mul(
                out[:, H:, :], E[:, H:, :],
                dinv[:, H:].unsqueeze(2).broadcast_to([P, NT - H, NO]))
            nc.sync.dma_start(y_view[:, H:, :], out[:, H:, :])

    nc.compile()
    return nc


_NC = None


def _get_nc():
    global _NC
    if _NC is None:
        _NC = _build()
    return _NC


def _host_params(scale1, rot1, scale2, rot2, scale3, rot3):
    c = max(max(float(scale1[0]), 0.0) * float(scale2[0]), 0.0) * float(scale3[0])
    rev = np.arange(K - 1, -1, -1)
    r1 = rot1[:K].astype(np.float64)
    r2 = rot2[:K].astype(np.float64)
    r3 = rot3[:K].astype(np.float64)
    a = np.pi / 2 + r1 + r2 + r3
    b = np.maximum(r2, 0.0) + r3
    # wrap A'/B by a common per-k multiple of 2pi (preserves the max branch
    # and sin values) so |max(A'-at, B)| < 3pi and one add_range_wrap wraps it
    s = TWO_PI * np.round((a + b) / 2.0 / TWO_PI)
    a -= s
    b -= s
    assert np.abs(a).max() + np.pi / 2 < 3 * np.pi - 0.2, "ARW bound violated"
    assert np.abs(b).max() < 3 * np.pi - 0.2, "ARW bound violated"
    row = np.zeros((PC_W,), np.float64)
    row[PC_C] = c
    row[PC_A:PC_A + K] = a[rev]
    row[PC_B:PC_B + K] = b[rev]
    return np.tile(row.astype(np.float32)[None, :], (P, 1))


def kernel(x, scale1, rot1, scale2, rot2, scale3, rot3, _trace=False):
    nc = _get_nc()
    pc = _host_params(scale1, rot1, scale2, rot2, scale3, rot3)
    x = np.ascontiguousarray(x, dtype=np.float32)
    xh = x.astype(np.float16)
    x9h = np.ascontiguousarray(x[:, 0:K])
    in_maps = [
        {"x": xh[c * ROWS:(c + 1) * ROWS], "pc": pc,
         "x9": x9h[c * ROWS:(c + 1) * ROWS]} for c in range(NCORES)
    ]
    res = run_bass_kernel_spmd(nc, in_maps, core_ids=list(range(NCORES)),
                               trace=_trace)
    out = np.concatenate([res.results[c]["y"] for c in range(NCORES)], axis=0)
    if _trace:
        return out, res
    return out


# revision 22
# speedup vs baseline: 1.1546x; 1.0578x over previous
"""Trainium2 Bass kernel for nn_Net_2491081031714.

Math per row x (784 f32):
  s_k = sum_{j>=k} x_j^2 (k=0..9), r = sqrt(s_0)
  theta_k = arccos(x_k / sqrt(s_k)) = pi/2 - arctan(x_k / sqrt(s_{k+1}))
  th_k = relu(relu(theta_k + rot1_k) + rot2_k) + rot3_k
       = max(theta_k + A_k, B_k),  A = rot1+rot2+rot3, B = max(rot2,0)+rot3
  r3 = r * relu(relu(scale1)*scale2)*scale3
  cart = polar_to_cartesian(r3, th)  (10 values); out = softmax(cart)

Only th_0..th_8 survive (polar_linear out_dim=10 truncates), so the heavy
work is the per-row sum of squares of cols 9..783, streamed in fp16 over
two DMA rings (Sync HWDGE + GpSimd SWDGE) and consumed tile-at-a-time by
the Scalar (ACT Square+accum) and Vector (STT+accum) engines, whose
accum-reads drop s9 straight into the suffix-scan's data1 slot 0.
rsqrt is a Quake seed + one Halley step; theta uses the exact arctan
identity; the relu chain folds to max(A'-arctan, B) with A'/B pre-wrapped
by a common 2pi multiple on the host so one add_range_wrap replaces the
3-op magic-rounding wrap; exp's table set is prefetched by a pinned dummy
ACT Exp so the switch overlaps the DVE tail.

Sharding: pure batch data-parallel over 8 cores (2048 rows each).
"""

import numpy as np

import concourse.bacc as bacc
import concourse.tile as tile
from concourse import mybir
from concourse.bass_utils import run_bass_kernel_spmd

AF = mybir.ActivationFunctionType
OP = mybir.AluOpType
F32 = mybir.dt.float32
I32 = mybir.dt.int32
F16 = mybir.dt.float16

B, N = 16384, 784
NCORES = 8
ROWS = B // NCORES          # 2048
P = 128
NT = ROWS // P              # 16 row-tiles per core
K = 9                       # thetas that matter
NO = 10                     # output classes

TWO_PI = 6.283185307179586
HALF_PI = 1.5707963267948966
RSQRT_MAGIC = 0x5F3759DF    # Quake rsqrt seed constant

# pc (host-prepared params) column layout
PC_C = 0                    # scale product
PC_A = 1                    # wrapped pi/2 + rot1+rot2+rot3, reversed k [9]
PC_B = PC_A + K             # wrapped max(rot2,0)+rot3, reversed k     [9]
PC_W = PC_B + K

# DMA groups (start_tile, n_tiles, ring): ring 0 = Sync HWDGE (~200 GB/s),
# ring 1 = GpSimd SWDGE (~137 GB/s).  3:2 interleave so the two rings'
# packet streams overlap and groups land roughly in tile order.
GROUPS = [(0, 2, 0), (2, 2, 1), (4, 2, 0), (6, 2, 0), (8, 2, 1), (10, 2, 0),
          (12, 2, 0), (14, 2, 1)]
# whole-tile engine assignment: measured DVE 1051ns/tile vs ACT 1222ns/tile
# (incl. accum-read) -> 9/7.  Queue order follows expected land order.
DVE_ORDER = (0, 1, 2, 4, 8, 6, 10, 14, 12)
ACT_ORDER = (3, 5, 9, 7, 11, 15, 13)


def _build():
    nc = bacc.Bacc("TRN2", target_bir_lowering=False, debug=False)
    x = nc.dram_tensor("x", [ROWS, N], F16, kind="ExternalInput")
    x9 = nc.dram_tensor("x9", [ROWS, K], F32, kind="ExternalInput")
    pc = nc.dram_tensor("pc", [P, PC_W], F32, kind="ExternalInput")
    y = nc.dram_tensor("y", [ROWS, NO], F32, kind="ExternalOutput")

    # row <-> (partition, slot) mapping: row = 16*p + t, so a group's rows
    # are contiguous per partition (up to 6272B descriptors)
    xg_view = x.rearrange("(p t) n -> p t n", p=P)              # [P, NT, N]
    x9_view = x9.rearrange("(p t) k -> p t k", p=P)             # [P, NT, K]
    y_view = y.rearrange("(p t) k -> p t k", p=P)               # [P, NT, NO]

    with tile.TileContext(nc) as tc:
        with (
            tc.tile_pool(name="xpool", bufs=1) as xpool,
            tc.tile_pool(name="sing", bufs=1) as sing,
        ):
            # ACT table preload: first ACTIVATE is a Sin so trig_and_small
            # (sin+arctan+square+relu) loads under the DMA ramp.
            warm = sing.tile([P, 1], F32)
            nc.vector.memset(warm[:], 0.0)
            nc.scalar.activation(warm[:], warm[:], AF.Sin)

            xg = [xpool.tile([P, nt, N], F16, name=f"xg{g}", tag=f"xg{g}")
                  for g, (t0, nt, ring) in enumerate(GROUPS)]
            pct = sing.tile([P, PC_W], F32)
            x9n = sing.tile([P, NT, K], F32)      # x[:, 0:9] natural order
            for g, (t0, nt, ring) in enumerate(GROUPS):
                eng = nc.sync if ring == 0 else nc.gpsimd
                eng.dma_start(xg[g][:], xg_view[:, t0:t0 + nt, :])
            # pc/x9 are tiny and first needed at ~20us (epilogue), so they go
            # behind all x groups on the slow SWDGE ring
            nc.gpsimd.dma_start(pct[:], pc[:])
            nc.gpsimd.dma_start(x9n[:], x9_view)

            # persistent small tiles
            d0s = sing.tile([P, NT, NO], F32)     # scan data0 for suffix sums
            d1s = sing.tile([P, NT, NO], F32)     # scan data1 for suffix sums
            scnb = sing.tile([P, NT, NO], F32)    # [0, sin_0..sin_8] per block
            d1p = sing.tile([P, NT, NO], F32)     # scan data1 for cumprod
            sqa = sing.tile([P, N - K], F16)      # ACT squares scratch (dead)
            sqd = sing.tile([P, N - K], F16)      # DVE squares scratch (dead)

            nc.gpsimd.memset(d0s[:], 1.0)
            nc.gpsimd.memset(d0s[:, :, 0:1], 0.0)
            nc.gpsimd.memset(scnb[:, :, 0:1], 0.0)
            nc.gpsimd.memset(d1p[:, :, 1:], 0.0)
            # squares of the first 9 cols (reversed order) on the idle gpsimd
            nc.gpsimd.tensor_mul(d1s[:, :, 1:NO], x9n[:, :, ::-1],
                                 x9n[:, :, ::-1])

            # ---- main streaming square+reduce: each tile goes whole to one
            #      engine; the accum-read drops s9 straight into the scan's
            #      data1 slot 0 (no separate seed add) ----
            tile_group = {}
            for g, (t0, nt, ring) in enumerate(GROUPS):
                for j in range(nt):
                    tile_group[t0 + j] = (g, j)
            for t in DVE_ORDER:
                g, j = tile_group[t]
                nc.vector.scalar_tensor_tensor(
                    out=sqd[:], in0=xg[g][:, j, K:N], scalar=1.0,
                    in1=xg[g][:, j, K:N], op0=OP.mult, op1=OP.mult,
                    accum_out=d1s[:, t, 0:1],
                )
            for t in ACT_ORDER:
                g, j = tile_group[t]
                nc.scalar.activation(
                    out=sqa[:], in_=xg[g][:, j, K:N], func=AF.Square,
                    accum_out=d1s[:, t, 0:1],
                )

            # ---- epilogue (batched over all 16 row-tiles) ----
            ep = sing

            # suffix-sum scan: S[:, :, m] = s_{9-m} for m=0..9 (m=9 -> s_0)
            S = ep.tile([P, NT, NO], F32)
            nc.vector.tensor_tensor_scan(
                out=S[:].rearrange("p b k -> p (b k)"),
                data0=d0s[:].rearrange("p b k -> p (b k)"),
                data1=d1s[:].rearrange("p b k -> p (b k)"),
                initial=0.0, op0=OP.mult, op1=OP.add,
            )

            # rsqrt of all 10 suffix sums: Quake seed + one Halley step
            # y1 = y0*(1.875 - 1.25*w + 0.375*w^2), w = S*y0^2  (~1e-4 rel)
            sbits = S[:].bitcast(I32)
            y0i = ep.tile([P, NT, NO], I32)
            nc.vector.tensor_scalar(out=y0i[:], in0=sbits, scalar1=1, scalar2=-1,
                                    op0=OP.arith_shift_right, op1=OP.bitwise_xor)
            nc.vector.tensor_scalar(out=y0i[:], in0=y0i[:],
                                    scalar1=RSQRT_MAGIC + 1, scalar2=None,
                                    op0=OP.add)
            yv = y0i[:].bitcast(F32)
            aa = ep.tile([P, NT, NO], F32)
            ww = ep.tile([P, NT, NO], F32)
            inv = ep.tile([P, NT, NO], F32)
            dacc = ep.tile([P, 1], F32)   # dummy accum for affine_mul_reduce
            nc.vector.tensor_mul(aa[:], yv, yv)
            nc.vector.tensor_mul(ww[:], aa[:], S[:])
            nc.vector.affine_mul_reduce(out=aa[:], accum_out=dacc[:],
                                        in0=ww[:], in1=ww[:], scale=0.375,
                                        bias=-1.25)
            nc.vector.affine_mul_reduce(out=inv[:], accum_out=dacc[:],
                                        in0=aa[:], in1=yv, scale=1.0,
                                        bias=1.875)
            # inv[:, :, m] = rsqrt(s_{9-m})

            # r3 = c * s_0 * rsqrt(s_0) -> cumprod scan seed, on gpsimd so it
            # overlaps the DVE arctan path
            nc.gpsimd.tensor_mul(d1p[:, :, 0:1], S[:, :, NO - 1:NO],
                                 inv[:, :, NO - 1:NO])
            nc.gpsimd.tensor_scalar(out=d1p[:, :, 0:1], in0=d1p[:, :, 0:1],
                                    scalar1=pct[:, PC_C:PC_C + 1],
                                    scalar2=None, op0=OP.mult)

            # theta_k = pi/2 - arctan(x_k * rsqrt(s_{k+1})); reversed order j
            # uses w_j = x_{8-j} * rsqrt(s_{9-j}) = x9n_rev * inv[:, :, 0:9]
            w9 = ep.tile([P, NT, K], F32)
            nc.vector.tensor_mul(w9[:], x9n[:, :, ::-1], inv[:, :, 0:K])
            at = ep.tile([P, NT, K], F32)
            nc.scalar.activation(at[:], w9[:], AF.Arctan)

            av = pct[:, PC_A:PC_A + K].unsqueeze(1).broadcast_to([P, NT, K])
            bv = pct[:, PC_B:PC_B + K].unsqueeze(1).broadcast_to([P, NT, K])

            th = ep.tile([P, NT, K], F32)
            # th3 = max(A' - arctan, B); A'/B are pre-wrapped by a common 2pi
            # multiple per k, so |th3| < 3pi and one range-wrap suffices
            nc.vector.scalar_tensor_tensor(out=th[:], in0=at[:], scalar=-1.0,
                                           in1=av, op0=OP.mult, op1=OP.add)
            nc.vector.tensor_tensor(out=th[:], in0=th[:], in1=bv, op=OP.max)
            thp = ep.tile([P, NT, K], F32)
            nc.vector.add_range_wrap(out=thp[:], in_=th[:], shift=0.0,
                                     bound=np.pi, period=TWO_PI)

            # sins in natural order into scnb slots 1..9 (thp is rev order)
            nc.scalar.activation(scnb[:, :, 1:NO], thp[:, :, ::-1], AF.Sin)
            # cos (rev order) = sin(wrap(thp + pi/2)) via add_range_wrap
            y2 = ep.tile([P, NT, K], F32)
            nc.vector.add_range_wrap(out=y2[:], in_=thp[:], shift=HALF_PI,
                                     bound=np.pi, period=TWO_PI)
            ccr = ep.tile([P, NT, K], F32)
            nc.scalar.activation(ccr[:], y2[:], AF.Sin)

            # prefetch the exp table set while the DVE runs the cumprod tail.
            # The input view of ccr pins this AFTER the sins in the schedule
            # (a free-floating dummy would be hoisted early and thrash tables).
            nc.scalar.activation(warm[:], ccr[:, 0, 0:1], AF.Exp)

            # cumprod scan: PP[:, :, m] = r3 * prod_{i<m} sin_i
            PP = ep.tile([P, NT, NO], F32)
            nc.vector.tensor_tensor_scan(
                out=PP[:].rearrange("p b k -> p (b k)"),
                data0=scnb[:].rearrange("p b k -> p (b k)"),
                data1=d1p[:].rearrange("p b k -> p (b k)"),
                initial=0.0, op0=OP.mult, op1=OP.add,
            )

            lg = ep.tile([P, NT, NO], F32)
            # the two small slot products go to gpsimd, in parallel with the
            # DVE's 8-slot product
            nc.gpsimd.tensor_mul(lg[:, :, 0:1], PP[:, :, K - 1:K],
                                 ccr[:, :, 0:1])
            nc.gpsimd.tensor_mul(lg[:, :, 1:2], PP[:, :, K - 1:K],
                                 scnb[:, :, NO - 1:NO])
            nc.vector.tensor_mul(lg[:, :, 2:NO], PP[:, :, 7::-1], ccr[:, :, 1:K])

            # softmax without max-sub (|logits| <= ~45, f32-safe)
            E = ep.tile([P, NT, NO], F32)
            nc.scalar.activation(E[:], lg[:], AF.Exp)
            ds = ep.tile([P, NT], F32)
            nc.vector.tensor_reduce(out=ds[:], in_=E[:], axis=mybir.AxisListType.X,
                                    op=OP.add)
            dinv = ep.tile([P, NT], F32)
            nc.vector.reciprocal_approx_fast(dinv[:], ds[:])
            out = ep.tile([P, NT, NO], F32)
            H = NT // 2
            nc.vector.tensor_mul(
                out[:, 0:H, :], E[:, 0:H, :],
                dinv[:, 0:H].unsqueeze(2).broadcast_to([P, H, NO]))
            nc.sync.dma_start(y_view[:, 0:H, :], out[:, 0:H, :])
            nc.vector.tensor_mul(
                out[:, H:, :], E[:, H:, :],
                dinv[:, H:].unsqueeze(2).broadcast_to([P, NT - H, NO]))
            nc.sync.dma_start(y_view[:, H:, :], out[:, H:, :])

    nc.compile()
    return nc


_NC = None


def _get_nc():
    global _NC
    if _NC is None:
        _NC = _build()
    return _NC


def _host_params(scale1, rot1, scale2, rot2, scale3, rot3):
    c = max(max(float(scale1[0]), 0.0) * float(scale2[0]), 0.0) * float(scale3[0])
    rev = np.arange(K - 1, -1, -1)
    r1 = rot1[:K].astype(np.float64)
    r2 = rot2[:K].astype(np.float64)
    r3 = rot3[:K].astype(np.float64)
    a = np.pi / 2 + r1 + r2 + r3
    b = np.maximum(r2, 0.0) + r3
    # wrap A'/B by a common per-k multiple of 2pi (preserves the max branch
    # and sin values) so |max(A'-at, B)| < 3pi and one add_range_wrap wraps it
    s = TWO_PI * np.round((a + b) / 2.0 / TWO_PI)
    a -= s
    b -= s
    assert np.abs(a).max() + np.pi / 2 < 3 * np.pi - 0.2, "ARW bound violated"
    assert np.abs(b).max() < 3 * np.pi - 0.2, "ARW bound violated"
    row = np.zeros((PC_W,), np.float64)
    row[PC_C] = c
    row[PC_A:PC_A + K] = a[rev]
    row[PC_B:PC_B + K] = b[rev]
    return np.tile(row.astype(np.float32)[None, :], (P, 1))


def kernel(x, scale1, rot1, scale2, rot2, scale3, rot3, _trace=False):
    nc = _get_nc()
    pc = _host_params(scale1, rot1, scale2, rot2, scale3, rot3)
    x = np.ascontiguousarray(x, dtype=np.float32)
    xh = x.astype(np.float16)
    x9h = np.ascontiguousarray(x[:, 0:K])
    in_maps = [
        {"x": xh[c * ROWS:(c + 1) * ROWS], "pc": pc,
         "x9": x9h[c * ROWS:(c + 1) * ROWS]} for c in range(NCORES)
    ]
    res = run_bass_kernel_spmd(nc, in_maps, core_ids=list(range(NCORES)),
                               trace=_trace)
    out = np.concatenate([res.results[c]["y"] for c in range(NCORES)], axis=0)
    if _trace:
        return out, res
    return out
